# revision 2
# baseline (speedup 1.0000x reference)
"""GPS layer (GraphGPS) Trainium2 kernel v2: ResGatedGraphConv + dense per-graph MHA + FFN.

Data-parallel over 64 graphs (8 graphs / 4096 nodes per core).  Edges sorted by
destination, bucketed into 128-dst windows.  x[src] rows are fetched with one
transposed bf16 dma_gather per window (feature-major), so q/v are computed on
the PE per edge-tile; k[dst] expansion and the segment-sum both run as one-hot
bf16 matmuls accumulating in PSUM.  Attention/FFN run per graph in bf16 with
all bias/BN algebra folded into column scales.
"""
import sys
sys.path.insert(0, '/opt/trn_rl_repo')
import numpy as np
import ml_dtypes
import concourse.bass as bass
import concourse.bacc as bacc
import concourse.tile as tile
from concourse import mybir
from concourse.bass_utils import run_bass_kernel_spmd
from concourse.masks import make_identity

N, E, D, H, B, S = 32768, 524288, 128, 4, 64, 512
HD = D // H          # 32
NC = 8               # cores
NPC = N // NC        # 4096 nodes per core
GPC = B // NC        # 8 graphs per core
WIN = 128            # dst window
NWIN = NPC // WIN    # 32 windows per core
WPG = S // WIN       # 4 windows per graph
EPS = 1e-5
F32 = mybir.dt.float32
BF16 = mybir.dt.bfloat16
I16 = mybir.dt.int16
P = 128
BF = ml_dtypes.bfloat16
GCH = 512        # gather chunk (idxs per dma_gather)
GSP = True       # single_packet


def _prep_edges(edge_index):
    src = np.asarray(edge_index[0], dtype=np.int64)
    dst = np.asarray(edge_index[1], dtype=np.int64)
    order = np.argsort(dst, kind='stable')
    ss, ds = src[order], dst[order]
    wid = ds // WIN                       # global window id, 0..255
    counts = np.bincount(wid, minlength=NC * NWIN)
    tpw = int(np.ceil(counts.max() / 128))   # tiles per window (uniform)
    cap = tpw * 128
    gsrc = np.zeros((NC * NWIN, cap), np.int16)        # pad 0 (valid node)
    ldst = np.full((NC * NWIN, cap), -1.0, np.float32)  # pad -1 (no one-hot)
    offs = np.zeros(NC * NWIN + 1, np.int64)
    np.cumsum(counts, out=offs[1:])
    for w in range(NC * NWIN):
        s, e = offs[w], offs[w + 1]
        n = e - s
        gsrc[w, :n] = ss[s:e]
        ldst[w, :n] = (ds[s:e] - w * WIN).astype(np.float32)
    gsrc = gsrc.reshape(NC, NWIN, cap)
    ldst = ldst.reshape(NC, NWIN, cap)
    return gsrc, ldst, tpw


def _build(nc, tpw):
    cap = tpw * 128
    capC = cap // 16
    xb = nc.declare_dram_parameter("xb", [N, P], BF16, isOutput=False)
    xlb = nc.declare_dram_parameter("xlb", [P, NPC], BF16, isOutput=False)
    gidx = nc.declare_dram_parameter("gidx", [P, NWIN * capC], I16, isOutput=False)
    ldw = nc.declare_dram_parameter("ldw", [P, NWIN * tpw], BF16, isOutput=False)
    ldstT = nc.declare_dram_parameter("ldstT", [NWIN, cap], BF16, isOutput=False)
    WresT = nc.declare_dram_parameter("WresT", [P, P], BF16, isOutput=False)
    WskipT = nc.declare_dram_parameter("WskipT", [P, P], BF16, isOutput=False)
    WkT = nc.declare_dram_parameter("WkT", [P, P], BF16, isOutput=False)
    WqvT = nc.declare_dram_parameter("WqvT", [P, 2 * P], BF16, isOutput=False)
    ipwT = nc.declare_dram_parameter("ipwT", [P, 3 * P], BF16, isOutput=False)
    opwT = nc.declare_dram_parameter("opwT", [P, P], BF16, isOutput=False)
    W1T = nc.declare_dram_parameter("W1T", [P, 2 * P], BF16, isOutput=False)
    W2a = nc.declare_dram_parameter("W2a", [P, P], BF16, isOutput=False)
    W2b = nc.declare_dram_parameter("W2b", [P, P], BF16, isOutput=False)
    kqb_d = nc.declare_dram_parameter("kqb", [P, P], BF16, isOutput=False)
    bvrep_d = nc.declare_dram_parameter("bvrep", [P, P], BF16, isOutput=False)
    cols = nc.declare_dram_parameter("cols", [P, 16], F32, isOutput=False)
    ipb = nc.declare_dram_parameter("ipb", [P, 3], F32, isOutput=False)
    b1c = nc.declare_dram_parameter("b1c", [P, 2], F32, isOutput=False)
    outT = nc.declare_dram_parameter("outT", [P, NPC], F32, isOutput=True)
    import os as _os
    DBG = _os.environ.get("K2_DBG", "0") == "1"
    if DBG:
        d_hin1 = nc.declare_dram_parameter("d_hin1", [P, NPC], BF16, isOutput=True)
        d_kv = nc.declare_dram_parameter("d_kv", [P, NWIN * 2 * P], BF16, isOutput=True)
        d_xgT = nc.declare_dram_parameter("d_xgT", [P, tpw * 128], BF16, isOutput=True)
        d_oT = nc.declare_dram_parameter("d_oT", [P, tpw * 128], BF16, isOutput=True)
        d_obig = nc.declare_dram_parameter("d_obig", [P, tpw * 128], BF16, isOutput=True)
        d_hloc = nc.declare_dram_parameter("d_hloc", [P, NPC], BF16, isOutput=True)
        d_ctx = nc.declare_dram_parameter("d_ctx", [P, 512], BF16, isOutput=True)
        d_sh2 = nc.declare_dram_parameter("d_sh2", [P, NPC], F32, isOutput=True)

    RSQ = float(1.0 / np.sqrt(HD))

    with tile.TileContext(nc) as tc:
        import contextlib
        with contextlib.ExitStack() as es:
            one = es.enter_context(tc.tile_pool(name="one", bufs=1))
            sb = es.enter_context(tc.tile_pool(name="sb", bufs=2))
            sb3 = es.enter_context(tc.tile_pool(name="sb3", bufs=3))
            ps = es.enter_context(tc.tile_pool(name="ps", bufs=2, space="PSUM"))
            psB = es.enter_context(tc.tile_pool(name="psB", bufs=2, space="PSUM"))
            psC = es.enter_context(tc.tile_pool(name="psC", bufs=1, space="PSUM"))

            # ---- static tiles ----
            identf = one.tile([P, P], F32)
            make_identity(nc, identf[:])
            iota_r = one.tile([P, P], mybir.dt.int32)
            nc.gpsimd.iota(iota_r[:], pattern=[[1, P]], base=0, channel_multiplier=0)
            iota_f = one.tile([P, P], BF16)
            nc.vector.tensor_copy(iota_f[:], iota_r[:])
            iota_cr = one.tile([P, 1], mybir.dt.int32)
            nc.gpsimd.iota(iota_cr[:], pattern=[[0, 1]], base=0, channel_multiplier=1)
            iota_c = one.tile([P, 1], BF16)
            nc.vector.tensor_copy(iota_c[:], iota_cr[:])
            ones_row = one.tile([1, P], BF16)
            nc.vector.memset(ones_row[:], 1.0)

            wresb = one.tile([P, P], BF16); nc.sync.dma_start(out=wresb[:], in_=WresT[:])
            wskipb = one.tile([P, P], BF16); nc.sync.dma_start(out=wskipb[:], in_=WskipT[:])
            wkb = one.tile([P, P], BF16); nc.sync.dma_start(out=wkb[:], in_=WkT[:])
            wqvb = one.tile([P, 2 * P], BF16); nc.sync.dma_start(out=wqvb[:], in_=WqvT[:])
            ipwb = one.tile([P, 3 * P], BF16); nc.sync.dma_start(out=ipwb[:], in_=ipwT[:])
            opwb = one.tile([P, P], BF16); nc.sync.dma_start(out=opwb[:], in_=opwT[:])
            w1b = one.tile([P, 2 * P], BF16); nc.sync.dma_start(out=w1b[:], in_=W1T[:])
            w2ab = one.tile([P, P], BF16); nc.sync.dma_start(out=w2ab[:], in_=W2a[:])
            w2bb = one.tile([P, P], BF16); nc.sync.dma_start(out=w2bb[:], in_=W2b[:])
            kqb = one.tile([P, P], BF16); nc.sync.dma_start(out=kqb[:], in_=kqb_d[:])
            bvrep = one.tile([P, P], BF16); nc.sync.dma_start(out=bvrep[:], in_=bvrep_d[:])
            colv = one.tile([P, 16], F32); nc.sync.dma_start(out=colv[:], in_=cols[:])
            ipbv = one.tile([P, 3], F32); nc.sync.dma_start(out=ipbv[:], in_=ipb[:])
            b1v = one.tile([P, 2], F32); nc.sync.dma_start(out=b1v[:], in_=b1c[:])
            c_bres, c_bskip = colv[:, 0:1], colv[:, 1:2]
            c_g1l, c_b1l = colv[:, 2:3], colv[:, 3:4]
            c_g1a, c_b1a = colv[:, 4:5], colv[:, 5:6]
            c_outb = colv[:, 6:7]
            c_g2, c_C2 = colv[:, 7:8], colv[:, 8:9]

            gidx_sb = one.tile([P, NWIN * capC], I16)
            nc.sync.dma_start(out=gidx_sb[:], in_=gidx[:])
            ldw_sb = one.tile([P, NWIN * tpw], BF16)
            nc.sync.dma_start(out=ldw_sb[:], in_=ldw[:])

            xlb_sb = one.tile([P, NPC], BF16)
            nc.sync.dma_start(out=xlb_sb[:], in_=xlb[:])

            hin1b = one.tile([P, NPC], BF16)
            SH2 = one.tile([P, NPC], F32)
            hloc = one.tile([P, NPC], BF16)
            kvtab = one.tile([P, NWIN * 2 * P], BF16)

            # ---- phase 1: tables ----
            for c in range(8):
                sl = slice(c * 512, (c + 1) * 512)
                pr = ps.tile([P, 512], F32, tag="b512")
                nc.tensor.matmul(pr[:], lhsT=wresb[:], rhs=xlb_sb[:, sl],
                                 start=True, stop=True)
                nc.scalar.activation(hin1b[:, sl], pr[:],
                                     mybir.ActivationFunctionType.Relu,
                                     bias=c_bres, scale=1.0)
                pr2 = ps.tile([P, 512], F32, tag="b512")
                nc.tensor.matmul(pr2[:], lhsT=wskipb[:], rhs=xlb_sb[:, sl],
                                 start=True, stop=True)
                skf = sb.tile([P, 512], F32, tag="skf")
                nc.scalar.activation(skf[:], pr2[:],
                                     mybir.ActivationFunctionType.Identity,
                                     bias=c_bskip, scale=1.0)
                tmp = sb.tile([P, 512], F32, tag="sh2t")
                nc.vector.tensor_add(out=tmp[:], in0=skf[:], in1=hin1b[:, sl])
                nc.vector.tensor_scalar(out=SH2[:, sl], in0=tmp[:],
                                        scalar1=c_g1l, scalar2=c_b1l,
                                        op0=mybir.AluOpType.mult,
                                        op1=mybir.AluOpType.add)
            for w in range(NWIN):
                pk = psB.tile([P, P], F32, tag="agg")
                nc.tensor.matmul(pk[:], lhsT=xlb_sb[:, w * P:(w + 1) * P],
                                 rhs=wkb[:], start=True, stop=True)
                nc.vector.tensor_add(out=kvtab[:, w * 2 * P:w * 2 * P + P],
                                     in0=pk[:], in1=kqb[:])
                nc.vector.tensor_copy(kvtab[:, w * 2 * P + P:(w + 1) * 2 * P],
                                      bvrep[:])

            # ---- main loop: per graph, attention first then its 4 windows ----
            npair = (tpw + 1) // 2
            for g in range(GPC):
                gs = slice(g * S, (g + 1) * S)
                hgb = hin1b[:, gs]
                # --- attention for graph g (depends only on hin1b) ---
                qkvb = []
                for j in range(3):
                    pq = ps.tile([P, 512], F32, tag="b512")
                    nc.tensor.matmul(pq[:], lhsT=ipwb[:, j * P:(j + 1) * P],
                                     rhs=hgb, start=True, stop=True)
                    qb = sb.tile([P, 512], BF16 if j < 2 else F32, tag=f"qkv{j}")
                    nc.scalar.activation(qb[:], pq[:],
                                         mybir.ActivationFunctionType.Identity,
                                         bias=ipbv[:, j:j + 1], scale=1.0)
                    qkvb.append(qb)
                ctxg = sb.tile([P, 512], BF16, tag="ctxg")
                vaugs = []
                for h in range(H):
                    va = sb.tile([P, 4 * (HD + 1)], BF16, tag=f"vaug{h}")
                    for c in range(4):
                        nc.vector.memset(
                            va[:, c * (HD + 1) + HD:(c + 1) * (HD + 1)], 1.0)
                    vaugs.append(va)
                for c in range(4):
                    pvT = psC.tile([P, P], F32, tag="trT")
                    nc.tensor.transpose(out=pvT[:],
                                        in_=qkvb[2][:, c * P:(c + 1) * P],
                                        identity=identf[:])
                    for h in range(H):
                        nc.vector.tensor_copy(
                            vaugs[h][:, c * (HD + 1):c * (HD + 1) + HD],
                            pvT[:, h * HD:(h + 1) * HD])
                for h in range(H):
                    hs = slice(h * HD, (h + 1) * HD)
                    vaug = vaugs[h]
                    if h * HD in (0, 32, 64):
                        kh, qh, khs = qkvb[1], qkvb[0], hs
                    else:
                        # PE matmul operands can only base at partition 0/32/64
                        kh = sb.tile([HD, 512], BF16, tag="kh3")
                        nc.vector.tensor_copy(kh[:], qkvb[1][hs, :])
                        qh = sb.tile([HD, 512], BF16, tag="qh3")
                        nc.vector.tensor_copy(qh[:], qkvb[0][hs, :])
                        khs = slice(0, HD)
                    pctx = psC.tile([HD + 1, 512], F32, tag="pctx")
                    for c in range(4):
                        psc = ps.tile([P, 512], F32, tag="b512")
                        nc.tensor.matmul(psc[:], lhsT=kh[khs, c * P:(c + 1) * P],
                                         rhs=qh[khs, :], start=True, stop=True)
                        esc = sb.tile([P, 512], BF16, tag="esc")
                        nc.scalar.activation(esc[:], psc[:],
                                             mybir.ActivationFunctionType.Exp,
                                             scale=RSQ)
                        nc.tensor.matmul(pctx[:], lhsT=vaug[:, c * (HD + 1):(c + 1) * (HD + 1)],
                                         rhs=esc[:], start=(c == 0), stop=(c == 3))
                    dcp = sb.tile([1, 512], F32, tag="dcp")
                    nc.vector.tensor_copy(dcp[:], pctx[HD:HD + 1, :])
                    rden = sb.tile([1, 512], F32, tag="rden")
                    nc.vector.reciprocal_approx_fast(out=rden[:], in_=dcp[:])
                    denb = sb.tile([HD, 512], F32, tag="denb")
                    nc.gpsimd.partition_broadcast(denb[:], rden[:])
                    nc.vector.tensor_mul(out=ctxg[hs, :], in0=pctx[:HD, :],
                                         in1=denb[:])
                if DBG and g == 0:
                    nc.sync.dma_start(out=d_ctx[:], in_=ctxg[:])
                pop = ps.tile([P, 512], F32, tag="b512")
                nc.tensor.matmul(pop[:], lhsT=opwb[:], rhs=ctxg[:],
                                 start=True, stop=True)
                aob = sb.tile([P, 512], F32, tag="aob")
                nc.scalar.activation(aob[:], pop[:],
                                     mybir.ActivationFunctionType.Identity,
                                     bias=c_outb, scale=1.0)
                uat = sb.tile([P, 512], F32, tag="uat")
                nc.vector.tensor_scalar(out=uat[:], in0=hgb,
                                        scalar1=c_g1a, scalar2=c_b1a,
                                        op0=mybir.AluOpType.mult,
                                        op1=mybir.AluOpType.add)

                # --- message passing: windows of graph g ---
                for wi in range(WPG):
                    w = g * WPG + wi
                    wsl = slice(w * P, (w + 1) * P)
                    ldsT = sb3.tile([1, cap], BF16, tag="ldsT")
                    nc.sync.dma_start(out=ldsT[:], in_=ldstT[w, None, :])
                    xgT = sb3.tile([P, cap], BF16, tag="xgT")
                    for c0 in range(0, cap, GCH):
                        gn = min(GCH, cap - c0)
                        nc.gpsimd.dma_gather(
                            out_ap=xgT[:, None, c0:c0 + gn], in_ap=xb[:, :],
                            idxs_ap=gidx_sb[:, w * capC + c0 // 16:
                                            w * capC + (c0 + gn) // 16],
                            num_idxs=gn, num_idxs_reg=gn, elem_size=P,
                            transpose=True, single_packet=GSP)
                    obig = sb.tile([P, cap], BF16, tag="obig")
                    nc.vector.tensor_tensor(
                        out=obig[:].rearrange("p (t n) -> p t n", t=tpw),
                        in0=ldw_sb[:, w * tpw:(w + 1) * tpw, None].to_broadcast([P, tpw, P]),
                        in1=iota_f[:, None, :].to_broadcast([P, tpw, P]),
                        op=mybir.AluOpType.is_equal)
                    oT = sb.tile([P, cap], BF16, tag="oT")
                    for b0 in range(0, cap, 512):
                        bn = min(512, cap - b0)
                        ldb = ps.tile([P, 512], F32, tag="b512")
                        nc.tensor.matmul(ldb[:, :bn], lhsT=ones_row[:],
                                         rhs=ldsT[:, b0:b0 + bn],
                                         start=True, stop=True)
                        nc.vector.tensor_tensor(
                            out=oT[:, b0:b0 + bn],
                            in0=iota_c[:].to_broadcast([P, bn]),
                            in1=ldb[:, :bn],
                            op=mybir.AluOpType.is_equal)
                    if DBG and w == 0:
                        nc.sync.dma_start(out=d_xgT[:], in_=xgT[:])
                        nc.sync.dma_start(out=d_oT[:], in_=oT[:])
                        nc.sync.dma_start(out=d_obig[:], in_=obig[:])
                    agg = psB.tile([P, P], F32, tag="agg")
                    kvw = kvtab[:, w * 2 * P:(w + 1) * 2 * P]
                    for pi in range(npair):
                        t0 = 2 * pi
                        nt = min(2, tpw - t0)
                        parg = ps.tile([P, 512], F32, tag="parg")
                        for ti in range(nt):
                            t = t0 + ti
                            osl = parg[:, ti * 256:(ti + 1) * 256]
                            nc.tensor.matmul(osl, lhsT=oT[:, t * P:(t + 1) * P],
                                             rhs=kvw, start=True, stop=False)
                            nc.tensor.matmul(osl, lhsT=xgT[:, t * P:(t + 1) * P],
                                             rhs=wqvb[:], start=False, stop=True)
                        pview = parg[:].rearrange("p (t x) -> p t x", t=2)
                        sig = sb.tile([P, 256], BF16, tag="sig")
                        sview = sig[:].rearrange("p (t x) -> p t x", t=2)
                        nc.scalar.activation(sview[:, :nt, :], pview[:, :nt, 0:P],
                                             mybir.ActivationFunctionType.Sigmoid)
                        msg = sb.tile([P, 256], BF16, tag="msg")
                        mview = msg[:].rearrange("p (t x) -> p t x", t=2)
                        nc.vector.tensor_mul(out=mview[:, :nt, :],
                                             in0=sview[:, :nt, :],
                                             in1=pview[:, :nt, P:2 * P])
                        for ti in range(nt):
                            t = t0 + ti
                            nc.tensor.matmul(agg[:], lhsT=obig[:, t * P:(t + 1) * P],
                                             rhs=msg[:, ti * P:(ti + 1) * P],
                                             start=(t == 0), stop=(t == tpw - 1))
                    asb = sb.tile([P, P], F32, tag="asb")
                    nc.vector.tensor_copy(asb[:], agg[:])
                    paT = psC.tile([P, P], F32, tag="trT")
                    nc.tensor.transpose(out=paT[:], in_=asb[:], identity=identf[:])
                    u2 = sb.tile([P, P], F32, tag="u2")
                    nc.vector.tensor_scalar_mul(u2[:], paT[:], c_g1l)
                    nc.vector.tensor_add(out=hloc[:, wsl], in0=u2[:], in1=SH2[:, wsl])

                # --- combine + FFN for graph g ---
                ph = sb.tile([P, 512], F32, tag="ph")
                nc.vector.tensor_add(out=ph[:], in0=uat[:], in1=hloc[:, gs])
                t1 = sb.tile([P, 512], F32, tag="t1")
                nc.vector.tensor_scalar_mul(t1[:], aob[:], c_g1a)
                hatb = sb.tile([P, 512], BF16, tag="hatb")
                nc.vector.tensor_add(out=hatb[:], in0=t1[:], in1=ph[:])
                ff = []
                for c in range(2):
                    pf = ps.tile([P, 512], F32, tag="b512")
                    nc.tensor.matmul(pf[:], lhsT=w1b[:, c * P:(c + 1) * P],
                                     rhs=hatb[:], start=True, stop=True)
                    ffc = sb.tile([P, 512], BF16, tag=f"ff{c}")
                    nc.scalar.activation(ffc[:], pf[:],
                                         mybir.ActivationFunctionType.Relu,
                                         bias=b1v[:, c:c + 1], scale=1.0)
                    ff.append(ffc)
                pf2 = ps.tile([P, 512], F32, tag="b512")
                nc.tensor.matmul(pf2[:], lhsT=w2ab[:], rhs=ff[0][:],
                                 start=True, stop=False)
                nc.tensor.matmul(pf2[:], lhsT=w2bb[:], rhs=ff[1][:],
                                 start=False, stop=True)
                t2 = sb.tile([P, 512], F32, tag="t2")
                nc.vector.tensor_add(out=t2[:], in0=pf2[:], in1=hatb[:])
                outf = sb.tile([P, 512], F32, tag="outf")
                nc.vector.tensor_scalar(out=outf[:], in0=t2[:],
                                        scalar1=c_g2, scalar2=c_C2,
                                        op0=mybir.AluOpType.mult,
                                        op1=mybir.AluOpType.add)
                nc.sync.dma_start(out=outT[:, gs], in_=outf[:])
            if DBG:
                nc.sync.dma_start(out=d_hin1[:], in_=hin1b[:])
                nc.sync.dma_start(out=d_kv[:], in_=kvtab[:])
                nc.sync.dma_start(out=d_hloc[:], in_=hloc[:])
                nc.sync.dma_start(out=d_sh2[:], in_=SH2[:])
    nc.compile()
    return nc


def kernel(x, edge_index, batch_ids, Wres, bres, Wk, bk, Wq, bq, Wv, bv,
           Wskip, bskip, g1l, b1l, g1a, b1a, in_proj_w, in_proj_b,
           out_proj_w, out_proj_b, W1, b1, W2, b2, g2, b2g):
    x = np.asarray(x, dtype=np.float32)
    gsrc, ldst, tpw = _prep_edges(np.asarray(edge_index))
    cap = tpw * 128
    capC = cap // 16
    bnf = 1.0 / np.sqrt(1.0 + EPS)

    xb = np.ascontiguousarray(x.astype(BF))                      # [N, 128]
    xT = np.ascontiguousarray(x.T.astype(BF))                    # [128, N]

    # gidx: [NC, 128, NWIN*capC] int16, idx i of window w at [i%16, w*capC+i//16]
    gidx = np.zeros((NC, P, NWIN * capC), np.int16)
    ldw = np.zeros((NC, P, NWIN * tpw), np.float32)
    for c in range(NC):
        gi = gsrc[c].reshape(NWIN, capC, 16)                     # i = col*16 + p
        g16 = np.transpose(gi, (2, 0, 1)).reshape(16, NWIN * capC)
        gidx[c] = np.tile(g16, (8, 1))
        ldw[c] = np.transpose(ldst[c].reshape(NWIN, tpw, P), (2, 0, 1)) \
            .reshape(P, NWIN * tpw)

    cols = np.zeros((128, 16), np.float32)
    cols[:, 0] = bres; cols[:, 1] = bskip
    cols[:, 2] = g1l * bnf; cols[:, 3] = b1l
    cols[:, 4] = g1a * bnf; cols[:, 5] = b1a
    cols[:, 6] = out_proj_b
    cols[:, 7] = g2 * bnf; cols[:, 8] = (g2 * bnf) * b2 + b2g

    common = dict(
        xb=xb,
        WresT=np.ascontiguousarray(Wres.T.astype(BF)),
        WskipT=np.ascontiguousarray(Wskip.T.astype(BF)),
        WkT=np.ascontiguousarray(Wk.T.astype(BF)),
        WqvT=np.ascontiguousarray(np.concatenate([Wq.T, Wv.T], axis=1).astype(BF)),
        ipwT=np.ascontiguousarray(in_proj_w.T.astype(BF)),
        opwT=np.ascontiguousarray(out_proj_w.T.astype(BF)),
        W1T=np.ascontiguousarray(W1.T.astype(BF)),
        W2a=np.ascontiguousarray(W2.T[:128].astype(BF)),
        W2b=np.ascontiguousarray(W2.T[128:].astype(BF)),
        kqb=np.ascontiguousarray(np.tile((bk + bq).astype(np.float32)[None, :],
                                         (128, 1)).astype(BF)),
        bvrep=np.ascontiguousarray(np.tile(np.asarray(bv, np.float32)[None, :],
                                           (128, 1)).astype(BF)),
        cols=cols,
        ipb=np.ascontiguousarray(np.asarray(in_proj_b, np.float32).reshape(3, 128).T),
        b1c=np.ascontiguousarray(np.asarray(b1, np.float32).reshape(2, 128).T),
    )
    in_maps = []
    for c in range(NC):
        m = dict(common)
        m["xlb"] = np.ascontiguousarray(xT[:, c * NPC:(c + 1) * NPC])
        m["gidx"] = np.ascontiguousarray(gidx[c])
        m["ldw"] = np.ascontiguousarray(ldw[c].astype(BF))
        m["ldstT"] = np.ascontiguousarray(
            np.where(ldst[c] < 0, 0.0, ldst[c]).astype(BF))
        in_maps.append(m)

    nc = bacc.Bacc("TRN2", target_bir_lowering=False, debug=False, num_devices=NC)
    _build(nc, tpw)
    res = run_bass_kernel_spmd(nc, in_maps, list(range(NC)))
    if getattr(res, "exec_time_ns", None):
        print(f"HW exec time: {res.exec_time_ns} ns")
    out = np.empty((N, D), np.float32)
    for c in range(NC):
        out[c * NPC:(c + 1) * NPC] = res.results[c]["outT"].T
    import os as _os
    if _os.environ.get("K2_DBG", "0") == "1":
        np.savez("/tmp/k2dbg.npz",
                 **{k + "_0": np.asarray(v) for k, v in res.results[0].items()},
                 gsrc0=gsrc[0], ldst0=ldst[0], tpw=tpw)
    return out


# revision 3
# speedup vs baseline: 1.3718x; 1.3718x over previous
"""GPS layer (GraphGPS) Trainium2 kernel v2: ResGatedGraphConv + dense per-graph MHA + FFN.

Data-parallel over 64 graphs (8 graphs / 4096 nodes per core).  Edges sorted by
destination, bucketed into 128-dst windows.  x[src] rows are fetched with one
transposed bf16 dma_gather per window (feature-major), so q/v are computed on
the PE per edge-tile; k[dst] expansion and the segment-sum both run as one-hot
bf16 matmuls accumulating in PSUM.  Attention/FFN run per graph in bf16 with
all bias/BN algebra folded into column scales.
"""
import sys
sys.path.insert(0, '/opt/trn_rl_repo')
import numpy as np
import ml_dtypes
import concourse.bass as bass
import concourse.bacc as bacc
import concourse.tile as tile
from concourse import mybir
from concourse.bass_utils import run_bass_kernel_spmd
from concourse.masks import make_identity

N, E, D, H, B, S = 32768, 524288, 128, 4, 64, 512
HD = D // H          # 32
NC = 8               # cores
NPC = N // NC        # 4096 nodes per core
GPC = B // NC        # 8 graphs per core
WIN = 128            # dst window
NWIN = NPC // WIN    # 32 windows per core
WPG = S // WIN       # 4 windows per graph
EPS = 1e-5
F32 = mybir.dt.float32
BF16 = mybir.dt.bfloat16
I16 = mybir.dt.int16
P = 128
BF = ml_dtypes.bfloat16
GCH = 512        # gather chunk (idxs per dma_gather)
GSP = True       # single_packet


def _prep_edges(edge_index):
    src = np.asarray(edge_index[0], dtype=np.int64)
    dst = np.asarray(edge_index[1], dtype=np.int64)
    order = np.argsort(dst, kind='stable')
    ss, ds = src[order], dst[order]
    wid = ds // WIN                       # global window id, 0..255
    counts = np.bincount(wid, minlength=NC * NWIN)
    tpw = int(np.ceil(counts.max() / 128))   # tiles per window (uniform)
    cap = tpw * 128
    gsrc = np.zeros((NC * NWIN, cap), np.int16)        # pad 0 (valid node)
    ldst = np.full((NC * NWIN, cap), -1.0, np.float32)  # pad -1 (no one-hot)
    offs = np.zeros(NC * NWIN + 1, np.int64)
    np.cumsum(counts, out=offs[1:])
    for w in range(NC * NWIN):
        s, e = offs[w], offs[w + 1]
        n = e - s
        gsrc[w, :n] = ss[s:e]
        ldst[w, :n] = (ds[s:e] - w * WIN).astype(np.float32)
    gsrc = gsrc.reshape(NC, NWIN, cap)
    ldst = ldst.reshape(NC, NWIN, cap)
    cw = counts.reshape(NC, NWIN).max(axis=0)
    capws = (np.maximum(np.ceil(cw / 128).astype(int), 1) * 128).tolist()
    return gsrc, ldst, tpw, capws


def _build(nc, tpw, capws):
    cap = tpw * 128
    capC = cap // 16
    xb = nc.declare_dram_parameter("xb", [N, P], BF16, isOutput=False)
    xlb = nc.declare_dram_parameter("xlb", [P, NPC], BF16, isOutput=False)
    gidx = nc.declare_dram_parameter("gidx", [P, NWIN * capC], I16, isOutput=False)
    ldw = nc.declare_dram_parameter("ldw", [P, NWIN * tpw], BF16, isOutput=False)
    ldstT = nc.declare_dram_parameter("ldstT", [NWIN, cap], BF16, isOutput=False)
    WresT = nc.declare_dram_parameter("WresT", [P, P], BF16, isOutput=False)
    WskipT = nc.declare_dram_parameter("WskipT", [P, P], BF16, isOutput=False)
    WkT = nc.declare_dram_parameter("WkT", [P, P], BF16, isOutput=False)
    WqvT = nc.declare_dram_parameter("WqvT", [P, 2 * P], BF16, isOutput=False)
    ipwT = nc.declare_dram_parameter("ipwT", [P, 3 * P], BF16, isOutput=False)
    opwT = nc.declare_dram_parameter("opwT", [P, P], BF16, isOutput=False)
    W1T = nc.declare_dram_parameter("W1T", [P, 2 * P], BF16, isOutput=False)
    W2a = nc.declare_dram_parameter("W2a", [P, P], BF16, isOutput=False)
    W2b = nc.declare_dram_parameter("W2b", [P, P], BF16, isOutput=False)
    kqb_d = nc.declare_dram_parameter("kqb", [P, P], BF16, isOutput=False)
    bvrep_d = nc.declare_dram_parameter("bvrep", [P, P], BF16, isOutput=False)
    cols = nc.declare_dram_parameter("cols", [P, 16], F32, isOutput=False)
    ipb = nc.declare_dram_parameter("ipb", [P, 3], F32, isOutput=False)
    b1c = nc.declare_dram_parameter("b1c", [P, 2], F32, isOutput=False)
    outT = nc.declare_dram_parameter("outT", [P, NPC], F32, isOutput=True)
    import os as _os
    DBG = _os.environ.get("K2_DBG", "0") == "1"
    if DBG:
        d_hin1 = nc.declare_dram_parameter("d_hin1", [P, NPC], BF16, isOutput=True)
        d_kv = nc.declare_dram_parameter("d_kv", [P, NWIN * 2 * P], BF16, isOutput=True)
        d_xgT = nc.declare_dram_parameter("d_xgT", [P, tpw * 128], BF16, isOutput=True)
        d_oT = nc.declare_dram_parameter("d_oT", [P, tpw * 128], BF16, isOutput=True)
        d_obig = nc.declare_dram_parameter("d_obig", [P, tpw * 128], BF16, isOutput=True)
        d_hloc = nc.declare_dram_parameter("d_hloc", [P, NPC], BF16, isOutput=True)
        d_ctx = nc.declare_dram_parameter("d_ctx", [P, 512], BF16, isOutput=True)
        d_sh2 = nc.declare_dram_parameter("d_sh2", [P, NPC], F32, isOutput=True)

    RSQ = float(1.0 / np.sqrt(HD))

    with tile.TileContext(nc) as tc:
        import contextlib
        with contextlib.ExitStack() as es:
            one = es.enter_context(tc.tile_pool(name="one", bufs=1))
            sb = es.enter_context(tc.tile_pool(name="sb", bufs=2))
            sb3 = es.enter_context(tc.tile_pool(name="sb3", bufs=3))
            ps = es.enter_context(tc.tile_pool(name="ps", bufs=2, space="PSUM"))
            psB = es.enter_context(tc.tile_pool(name="psB", bufs=2, space="PSUM"))
            psC = es.enter_context(tc.tile_pool(name="psC", bufs=1, space="PSUM"))

            # ---- static tiles ----
            identf = one.tile([P, P], F32)
            make_identity(nc, identf[:])
            iota_r = one.tile([P, P], mybir.dt.int32)
            nc.gpsimd.iota(iota_r[:], pattern=[[1, P]], base=0, channel_multiplier=0)
            iota_f = one.tile([P, P], BF16)
            nc.vector.tensor_copy(iota_f[:], iota_r[:])
            iota_cr = one.tile([P, 1], mybir.dt.int32)
            nc.gpsimd.iota(iota_cr[:], pattern=[[0, 1]], base=0, channel_multiplier=1)
            iota_c = one.tile([P, 1], BF16)
            nc.vector.tensor_copy(iota_c[:], iota_cr[:])
            ones_row = one.tile([1, P], BF16)
            nc.vector.memset(ones_row[:], 1.0)

            wresb = one.tile([P, P], BF16); nc.sync.dma_start(out=wresb[:], in_=WresT[:])
            wskipb = one.tile([P, P], BF16); nc.sync.dma_start(out=wskipb[:], in_=WskipT[:])
            wkb = one.tile([P, P], BF16); nc.sync.dma_start(out=wkb[:], in_=WkT[:])
            wqvb = one.tile([P, 2 * P], BF16); nc.sync.dma_start(out=wqvb[:], in_=WqvT[:])
            ipwb = one.tile([P, 3 * P], BF16); nc.sync.dma_start(out=ipwb[:], in_=ipwT[:])
            opwb = one.tile([P, P], BF16); nc.sync.dma_start(out=opwb[:], in_=opwT[:])
            w1b = one.tile([P, 2 * P], BF16); nc.sync.dma_start(out=w1b[:], in_=W1T[:])
            w2ab = one.tile([P, P], BF16); nc.sync.dma_start(out=w2ab[:], in_=W2a[:])
            w2bb = one.tile([P, P], BF16); nc.sync.dma_start(out=w2bb[:], in_=W2b[:])
            kqb = one.tile([P, P], BF16); nc.sync.dma_start(out=kqb[:], in_=kqb_d[:])
            bvrep = one.tile([P, P], BF16); nc.sync.dma_start(out=bvrep[:], in_=bvrep_d[:])
            colv = one.tile([P, 16], F32); nc.sync.dma_start(out=colv[:], in_=cols[:])
            ipbv = one.tile([P, 3], F32); nc.sync.dma_start(out=ipbv[:], in_=ipb[:])
            b1v = one.tile([P, 2], F32); nc.sync.dma_start(out=b1v[:], in_=b1c[:])
            c_bres, c_bskip = colv[:, 0:1], colv[:, 1:2]
            c_g1l, c_b1l = colv[:, 2:3], colv[:, 3:4]
            c_g1a, c_b1a = colv[:, 4:5], colv[:, 5:6]
            c_outb = colv[:, 6:7]
            c_g2, c_C2 = colv[:, 7:8], colv[:, 8:9]

            gidx_sb = one.tile([P, NWIN * capC], I16)
            nc.sync.dma_start(out=gidx_sb[:], in_=gidx[:])
            ldw_sb = one.tile([P, NWIN * tpw], BF16)
            nc.sync.dma_start(out=ldw_sb[:], in_=ldw[:])

            xlb_sb = one.tile([P, NPC], BF16)
            nc.sync.dma_start(out=xlb_sb[:], in_=xlb[:])

            hin1b = one.tile([P, NPC], BF16)
            SH2 = one.tile([P, NPC], F32)
            hloc = one.tile([P, NPC], BF16)
            kvtab = one.tile([P, NWIN * 2 * P], BF16)

            # ---- phase 1: tables ----
            for c in range(8):
                sl = slice(c * 512, (c + 1) * 512)
                pr = ps.tile([P, 512], F32, tag="b512")
                nc.tensor.matmul(pr[:], lhsT=wresb[:], rhs=xlb_sb[:, sl],
                                 start=True, stop=True)
                nc.scalar.activation(hin1b[:, sl], pr[:],
                                     mybir.ActivationFunctionType.Relu,
                                     bias=c_bres, scale=1.0)
                pr2 = ps.tile([P, 512], F32, tag="b512")
                nc.tensor.matmul(pr2[:], lhsT=wskipb[:], rhs=xlb_sb[:, sl],
                                 start=True, stop=True)
                skf = sb.tile([P, 512], F32, tag="skf")
                nc.scalar.activation(skf[:], pr2[:],
                                     mybir.ActivationFunctionType.Identity,
                                     bias=c_bskip, scale=1.0)
                tmp = sb.tile([P, 512], F32, tag="sh2t")
                nc.vector.tensor_add(out=tmp[:], in0=skf[:], in1=hin1b[:, sl])
                nc.vector.tensor_scalar(out=SH2[:, sl], in0=tmp[:],
                                        scalar1=c_g1l, scalar2=c_b1l,
                                        op0=mybir.AluOpType.mult,
                                        op1=mybir.AluOpType.add)
            for w in range(NWIN):
                pk = psB.tile([P, P], F32, tag="agg")
                nc.tensor.matmul(pk[:], lhsT=xlb_sb[:, w * P:(w + 1) * P],
                                 rhs=wkb[:], start=True, stop=True)
                nc.vector.tensor_add(out=kvtab[:, w * 2 * P:w * 2 * P + P],
                                     in0=pk[:], in1=kqb[:])
                nc.vector.tensor_copy(kvtab[:, w * 2 * P + P:(w + 1) * 2 * P],
                                      bvrep[:])

            # ---- main loop: per graph, attention first then its 4 windows ----
            npair = (tpw + 1) // 2
            for g in range(GPC):
                gs = slice(g * S, (g + 1) * S)
                hgb = hin1b[:, gs]
                # --- attention for graph g (depends only on hin1b) ---
                qkvb = []
                for j in range(3):
                    pq = ps.tile([P, 512], F32, tag="b512")
                    nc.tensor.matmul(pq[:], lhsT=ipwb[:, j * P:(j + 1) * P],
                                     rhs=hgb, start=True, stop=True)
                    qb = sb.tile([P, 512], BF16 if j < 2 else F32, tag=f"qkv{j}")
                    nc.scalar.activation(qb[:], pq[:],
                                         mybir.ActivationFunctionType.Identity,
                                         bias=ipbv[:, j:j + 1], scale=1.0)
                    qkvb.append(qb)
                ctxg = sb.tile([P, 512], BF16, tag="ctxg")
                vaugs = []
                for h in range(H):
                    va = sb.tile([P, 4 * (HD + 1)], BF16, tag=f"vaug{h}")
                    for c in range(4):
                        nc.vector.memset(
                            va[:, c * (HD + 1) + HD:(c + 1) * (HD + 1)], 1.0)
                    vaugs.append(va)
                for c in range(4):
                    pvT = psC.tile([P, P], F32, tag="trT")
                    nc.tensor.transpose(out=pvT[:],
                                        in_=qkvb[2][:, c * P:(c + 1) * P],
                                        identity=identf[:])
                    for h in range(H):
                        nc.vector.tensor_copy(
                            vaugs[h][:, c * (HD + 1):c * (HD + 1) + HD],
                            pvT[:, h * HD:(h + 1) * HD])
                for h in range(H):
                    hs = slice(h * HD, (h + 1) * HD)
                    vaug = vaugs[h]
                    if h * HD in (0, 32, 64):
                        kh, qh, khs = qkvb[1], qkvb[0], hs
                    else:
                        # PE matmul operands can only base at partition 0/32/64
                        kh = sb.tile([HD, 512], BF16, tag="kh3")
                        nc.vector.tensor_copy(kh[:], qkvb[1][hs, :])
                        qh = sb.tile([HD, 512], BF16, tag="qh3")
                        nc.vector.tensor_copy(qh[:], qkvb[0][hs, :])
                        khs = slice(0, HD)
                    pctx = psC.tile([HD + 1, 512], F32, tag="pctx")
                    for c in range(4):
                        psc = ps.tile([P, 512], F32, tag="b512")
                        nc.tensor.matmul(psc[:], lhsT=kh[khs, c * P:(c + 1) * P],
                                         rhs=qh[khs, :], start=True, stop=True)
                        esc = sb.tile([P, 512], BF16, tag="esc")
                        nc.scalar.activation(esc[:], psc[:],
                                             mybir.ActivationFunctionType.Exp,
                                             scale=RSQ)
                        nc.tensor.matmul(pctx[:], lhsT=vaug[:, c * (HD + 1):(c + 1) * (HD + 1)],
                                         rhs=esc[:], start=(c == 0), stop=(c == 3))
                    dcp = sb.tile([1, 512], F32, tag="dcp")
                    nc.vector.tensor_copy(dcp[:], pctx[HD:HD + 1, :])
                    rden = sb.tile([1, 512], F32, tag="rden")
                    nc.vector.reciprocal_approx_fast(out=rden[:], in_=dcp[:])
                    rdb = sb.tile([1, 512], BF16, tag="rdb")
                    nc.vector.tensor_copy(rdb[:], rden[:])
                    pdb = ps.tile([P, 512], F32, tag="b512")
                    nc.tensor.matmul(pdb[:HD, :], lhsT=ones_row[:, :HD],
                                     rhs=rdb[:], start=True, stop=True)
                    denb = sb.tile([HD, 512], F32, tag="denb")
                    nc.vector.tensor_copy(denb[:], pdb[:HD, :])
                    nc.vector.tensor_mul(out=ctxg[hs, :], in0=pctx[:HD, :],
                                         in1=denb[:])
                if DBG and g == 0:
                    nc.sync.dma_start(out=d_ctx[:], in_=ctxg[:])
                pop = ps.tile([P, 512], F32, tag="b512")
                nc.tensor.matmul(pop[:], lhsT=opwb[:], rhs=ctxg[:],
                                 start=True, stop=True)
                aob = sb.tile([P, 512], F32, tag="aob")
                nc.scalar.activation(aob[:], pop[:],
                                     mybir.ActivationFunctionType.Identity,
                                     bias=c_outb, scale=1.0)
                uat = sb.tile([P, 512], F32, tag="uat")
                nc.vector.tensor_scalar(out=uat[:], in0=hgb,
                                        scalar1=c_g1a, scalar2=c_b1a,
                                        op0=mybir.AluOpType.mult,
                                        op1=mybir.AluOpType.add)

                # --- message passing: windows of graph g ---
                for wi in range(WPG):
                    w = g * WPG + wi
                    wsl = slice(w * P, (w + 1) * P)
                    capw = capws[w]
                    ntile = capw // 128
                    ldsT = sb3.tile([1, cap], BF16, tag="ldsT")
                    nc.sync.dma_start(out=ldsT[:], in_=ldstT[w, None, :])
                    xgT = sb3.tile([P, cap], BF16, tag="xgT")
                    for c0 in range(0, capw, GCH):
                        gn = min(GCH, capw - c0)
                        nc.gpsimd.dma_gather(
                            out_ap=xgT[:, None, c0:c0 + gn], in_ap=xb[:, :],
                            idxs_ap=gidx_sb[:, w * capC + c0 // 16:
                                            w * capC + (c0 + gn) // 16],
                            num_idxs=gn, num_idxs_reg=gn, elem_size=P,
                            transpose=True, single_packet=GSP)
                    obig = sb.tile([P, cap], BF16, tag="obig")
                    nc.vector.tensor_tensor(
                        out=obig[:, :capw].rearrange("p (t n) -> p t n", t=ntile),
                        in0=ldw_sb[:, w * tpw:w * tpw + ntile, None].to_broadcast([P, ntile, P]),
                        in1=iota_f[:, None, :].to_broadcast([P, ntile, P]),
                        op=mybir.AluOpType.is_equal)
                    oT = sb.tile([P, cap], BF16, tag="oT")
                    for b0 in range(0, capw, 512):
                        bn = min(512, capw - b0)
                        ldb = ps.tile([P, 512], F32, tag="b512")
                        nc.tensor.matmul(ldb[:, :bn], lhsT=ones_row[:],
                                         rhs=ldsT[:, b0:b0 + bn],
                                         start=True, stop=True)
                        nc.vector.tensor_tensor(
                            out=oT[:, b0:b0 + bn],
                            in0=iota_c[:].to_broadcast([P, bn]),
                            in1=ldb[:, :bn],
                            op=mybir.AluOpType.is_equal)
                    if DBG and w == 0:
                        nc.sync.dma_start(out=d_xgT[:], in_=xgT[:])
                        nc.sync.dma_start(out=d_oT[:], in_=oT[:])
                        nc.sync.dma_start(out=d_obig[:], in_=obig[:])
                    agg = psB.tile([P, P], F32, tag="agg")
                    kvw = kvtab[:, w * 2 * P:(w + 1) * 2 * P]
                    for pi in range((ntile + 1) // 2):
                        t0 = 2 * pi
                        nt = min(2, ntile - t0)
                        parg = ps.tile([P, 512], F32, tag="parg")
                        for ti in range(nt):
                            t = t0 + ti
                            osl = parg[:, ti * 256:(ti + 1) * 256]
                            nc.tensor.matmul(osl, lhsT=oT[:, t * P:(t + 1) * P],
                                             rhs=kvw, start=True, stop=False)
                            nc.tensor.matmul(osl, lhsT=xgT[:, t * P:(t + 1) * P],
                                             rhs=wqvb[:], start=False, stop=True)
                        pview = parg[:].rearrange("p (t x) -> p t x", t=2)
                        sig = sb.tile([P, 256], BF16, tag="sig")
                        sview = sig[:].rearrange("p (t x) -> p t x", t=2)
                        nc.scalar.activation(sview[:, :nt, :], pview[:, :nt, 0:P],
                                             mybir.ActivationFunctionType.Sigmoid)
                        msg = sb.tile([P, 256], BF16, tag="msg")
                        mview = msg[:].rearrange("p (t x) -> p t x", t=2)
                        nc.vector.tensor_mul(out=mview[:, :nt, :],
                                             in0=sview[:, :nt, :],
                                             in1=pview[:, :nt, P:2 * P])
                        for ti in range(nt):
                            t = t0 + ti
                            nc.tensor.matmul(agg[:], lhsT=obig[:, t * P:(t + 1) * P],
                                             rhs=msg[:, ti * P:(ti + 1) * P],
                                             start=(t == 0), stop=(t == ntile - 1))
                    asb = sb.tile([P, P], F32, tag="asb")
                    nc.vector.tensor_copy(asb[:], agg[:])
                    paT = psC.tile([P, P], F32, tag="trT")
                    nc.tensor.transpose(out=paT[:], in_=asb[:], identity=identf[:])
                    u2 = sb.tile([P, P], F32, tag="u2")
                    nc.vector.tensor_scalar_mul(u2[:], paT[:], c_g1l)
                    nc.vector.tensor_add(out=hloc[:, wsl], in0=u2[:], in1=SH2[:, wsl])

                # --- combine + FFN for graph g ---
                ph = sb.tile([P, 512], F32, tag="ph")
                nc.vector.tensor_add(out=ph[:], in0=uat[:], in1=hloc[:, gs])
                t1 = sb.tile([P, 512], F32, tag="t1")
                nc.vector.tensor_scalar_mul(t1[:], aob[:], c_g1a)
                hatb = sb.tile([P, 512], BF16, tag="hatb")
                nc.vector.tensor_add(out=hatb[:], in0=t1[:], in1=ph[:])
                ff = []
                for c in range(2):
                    pf = ps.tile([P, 512], F32, tag="b512")
                    nc.tensor.matmul(pf[:], lhsT=w1b[:, c * P:(c + 1) * P],
                                     rhs=hatb[:], start=True, stop=True)
                    ffc = sb.tile([P, 512], BF16, tag=f"ff{c}")
                    nc.scalar.activation(ffc[:], pf[:],
                                         mybir.ActivationFunctionType.Relu,
                                         bias=b1v[:, c:c + 1], scale=1.0)
                    ff.append(ffc)
                pf2 = ps.tile([P, 512], F32, tag="b512")
                nc.tensor.matmul(pf2[:], lhsT=w2ab[:], rhs=ff[0][:],
                                 start=True, stop=False)
                nc.tensor.matmul(pf2[:], lhsT=w2bb[:], rhs=ff[1][:],
                                 start=False, stop=True)
                t2 = sb.tile([P, 512], F32, tag="t2")
                nc.vector.tensor_add(out=t2[:], in0=pf2[:], in1=hatb[:])
                outf = sb.tile([P, 512], F32, tag="outf")
                nc.vector.tensor_scalar(out=outf[:], in0=t2[:],
                                        scalar1=c_g2, scalar2=c_C2,
                                        op0=mybir.AluOpType.mult,
                                        op1=mybir.AluOpType.add)
                nc.sync.dma_start(out=outT[:, gs], in_=outf[:])
            if DBG:
                nc.sync.dma_start(out=d_hin1[:], in_=hin1b[:])
                nc.sync.dma_start(out=d_kv[:], in_=kvtab[:])
                nc.sync.dma_start(out=d_hloc[:], in_=hloc[:])
                nc.sync.dma_start(out=d_sh2[:], in_=SH2[:])
    nc.compile()
    return nc


def kernel(x, edge_index, batch_ids, Wres, bres, Wk, bk, Wq, bq, Wv, bv,
           Wskip, bskip, g1l, b1l, g1a, b1a, in_proj_w, in_proj_b,
           out_proj_w, out_proj_b, W1, b1, W2, b2, g2, b2g):
    x = np.asarray(x, dtype=np.float32)
    gsrc, ldst, tpw, capws = _prep_edges(np.asarray(edge_index))
    cap = tpw * 128
    capC = cap // 16
    bnf = 1.0 / np.sqrt(1.0 + EPS)

    xb = np.ascontiguousarray(x.astype(BF))                      # [N, 128]
    xT = np.ascontiguousarray(x.T.astype(BF))                    # [128, N]

    # gidx: [NC, 128, NWIN*capC] int16, idx i of window w at [i%16, w*capC+i//16]
    gidx = np.zeros((NC, P, NWIN * capC), np.int16)
    ldw = np.zeros((NC, P, NWIN * tpw), np.float32)
    for c in range(NC):
        gi = gsrc[c].reshape(NWIN, capC, 16)                     # i = col*16 + p
        g16 = np.transpose(gi, (2, 0, 1)).reshape(16, NWIN * capC)
        gidx[c] = np.tile(g16, (8, 1))
        ldw[c] = np.transpose(ldst[c].reshape(NWIN, tpw, P), (2, 0, 1)) \
            .reshape(P, NWIN * tpw)

    cols = np.zeros((128, 16), np.float32)
    cols[:, 0] = bres; cols[:, 1] = bskip
    cols[:, 2] = g1l * bnf; cols[:, 3] = b1l
    cols[:, 4] = g1a * bnf; cols[:, 5] = b1a
    cols[:, 6] = out_proj_b
    cols[:, 7] = g2 * bnf; cols[:, 8] = (g2 * bnf) * b2 + b2g

    common = dict(
        xb=xb,
        WresT=np.ascontiguousarray(Wres.T.astype(BF)),
        WskipT=np.ascontiguousarray(Wskip.T.astype(BF)),
        WkT=np.ascontiguousarray(Wk.T.astype(BF)),
        WqvT=np.ascontiguousarray(np.concatenate([Wq.T, Wv.T], axis=1).astype(BF)),
        ipwT=np.ascontiguousarray(in_proj_w.T.astype(BF)),
        opwT=np.ascontiguousarray(out_proj_w.T.astype(BF)),
        W1T=np.ascontiguousarray(W1.T.astype(BF)),
        W2a=np.ascontiguousarray(W2.T[:128].astype(BF)),
        W2b=np.ascontiguousarray(W2.T[128:].astype(BF)),
        kqb=np.ascontiguousarray(np.tile((bk + bq).astype(np.float32)[None, :],
                                         (128, 1)).astype(BF)),
        bvrep=np.ascontiguousarray(np.tile(np.asarray(bv, np.float32)[None, :],
                                           (128, 1)).astype(BF)),
        cols=cols,
        ipb=np.ascontiguousarray(np.asarray(in_proj_b, np.float32).reshape(3, 128).T),
        b1c=np.ascontiguousarray(np.asarray(b1, np.float32).reshape(2, 128).T),
    )
    in_maps = []
    for c in range(NC):
        m = dict(common)
        m["xlb"] = np.ascontiguousarray(xT[:, c * NPC:(c + 1) * NPC])
        m["gidx"] = np.ascontiguousarray(gidx[c])
        m["ldw"] = np.ascontiguousarray(ldw[c].astype(BF))
        m["ldstT"] = np.ascontiguousarray(
            np.where(ldst[c] < 0, 0.0, ldst[c]).astype(BF))
        in_maps.append(m)

    nc = bacc.Bacc("TRN2", target_bir_lowering=False, debug=False, num_devices=NC)
    _build(nc, tpw, capws)
    res = run_bass_kernel_spmd(nc, in_maps, list(range(NC)))
    if getattr(res, "exec_time_ns", None):
        print(f"HW exec time: {res.exec_time_ns} ns")
    out = np.empty((N, D), np.float32)
    for c in range(NC):
        out[c * NPC:(c + 1) * NPC] = res.results[c]["outT"].T
    import os as _os
    if _os.environ.get("K2_DBG", "0") == "1":
        np.savez("/tmp/k2dbg.npz",
                 **{k + "_0": np.asarray(v) for k, v in res.results[0].items()},
                 gsrc0=gsrc[0], ldst0=ldst[0], tpw=tpw)
    return out
